# revision 13
# baseline (speedup 1.0000x reference)
"""Causal multi-head attention (B=2, T=2048, DIM=2048, H=16, HD=128) on 8
Trainium2 NeuronCores.

Sharding: core = 4*b + g  (b = batch 0..1, g = head-group 0..3, 4 heads each).
Each core computes, for its batch b and heads 4g..4g+3:
  QKV projection -> causal attention -> partial out = attn_out @ wo[rows of g]
The host sums the 4 partial outputs per batch (the "all-reduce after wo").

On-device layout avoids every transpose:
  - host passes x[b].T, so projections contract d with d on partitions
  - Q^T/K^T kept as [hd, t] (head dim on partitions): K_h at qkt slot 2h,
    Q_h at slot 2h+1
  - scores computed as S^T = K^T_tile.T @ Q^T  ([j, i] layout)
  - exp via ScalarE; causal masking = multiply diagonal tiles by 0/1 masks
  - diagonal score tiles are NARROWED: only query columns >= key-tile start
    are computed/exp'd/masked (the rest is fully masked anyway)
  - P@V computed as O^T via lhsT = V tile (natural [t, hd] layout)
  - denominator via ones-vector matmul over in-place pair sums on DVE,
    lagged one pair so the add latency stays off the in-order PE
  - normalization via approx reciprocal + GPSIMD partition-broadcast of 1/d
    + VectorE multiply
  - wo projection consumes O^T tiles directly as stationary operands;
    chunk order 1,2,3,0 so every chunk's wo chains interleave into the next
    chunk's score/PV stream and the tail drain is short
Projection streams d-tiles in groups [2,3,4,4,3] (compute starts after two
d-tiles; group-0 PSUM->SBUF copies alternate DVE/ScalarE so dq=2 stays
PE-bound); per d-tile DMA order is wv, xT, wqk so the first V chain starts
earliest. masks/wo loads ride the scalar HWDGE ring, issued between groups
0 and 1 so they never contend with the first d-tiles or attention.

Weights/activations stream as bfloat16 (PE runs bf16 at the same full rate
as float32r; inputs/outputs of every matmul keep consistent dtype; psum,
denominators, reciprocal, and the final output stay fp32). End-to-end rel
err ~1e-3 vs the 2e-2 gate. ATTN_BF16=0 falls back to float32r.
"""

import math
import os

import numpy as np

B, T, D, H, HD = 2, 2048, 2048, 16, 128
NH = 4            # heads per core
NCORES = 8
TCH = 512         # query-chunk width (moving-operand free size)
NDT = D // 128    # 16 d-tiles (contraction tiles for projections)
NTT = T // 128    # 16 t-tiles
NCH = T // TCH    # 4 query chunks

BF16 = os.environ.get("ATTN_BF16", "0") not in ("", "0")

_BUILT = {}
LAST_RESULTS = None  # BassKernelResults of the most recent kernel() call


def _build(causal: bool):
    import concourse.mybir as mybir
    import concourse.tile as tile
    from concourse import bacc

    F32 = mybir.dt.float32
    DT = mybir.dt.bfloat16 if BF16 else mybir.dt.float32r
    EXP = mybir.ActivationFunctionType.Exp
    scale = 1.0 / math.sqrt(HD)

    nc = bacc.Bacc(None, name="attn")
    xT = nc.dram_tensor("xT", [D, T], DT, kind="ExternalInput")
    wqkv = nc.dram_tensor("wqkv", [D, 3 * NH * HD], DT, kind="ExternalInput")
    wo = nc.dram_tensor("wo", [NH * HD, D], DT, kind="ExternalInput")
    masks = nc.dram_tensor("masks", [128, 4 * TCH], DT, kind="ExternalInput")
    if not causal:
        maskT = nc.dram_tensor("maskT", [T, T], DT, kind="ExternalInput")
    out = nc.dram_tensor("out", [T, D], F32, kind="ExternalOutput")

    with tile.TileContext(nc) as tc:
        with tc.tile_pool(name="persist", bufs=1) as persist:
            # persistent operands for the attention phase
            qkt = persist.tile([128, 8, T], DT)           # slot 2h: K_h, 2h+1: Q_h
            vsb = persist.tile([128, NTT, NH * HD], DT)   # V, [t-tile][local t, head*hd]
            msb = persist.tile([128, 4 * TCH], DT)        # diagonal causal masks
            ones_f = persist.tile([128, 1], F32)
            ones = persist.tile([128, 1], DT)
            nc.vector.memset(ones_f[:], 1.0)
            nc.vector.tensor_copy(ones[:], ones_f[:])
            # dummy broadcast: preload the GpSimd PartitionBroadcast ucode
            # library now (~11us HBM fetch) so the first real normalize
            # doesn't stall the whole attention pipeline on LIBRARY_RELOAD
            warm = persist.tile([128, 1], F32)
            nc.gpsimd.partition_broadcast(warm[:], ones_f[0:1, :])

            # ---- Phase A: QKV projections, streaming x^T / wqkv d-tiles.
            # groups[0]=2 so compute starts after ~2.5MB of DMA; its
            # PSUM->SBUF copies split across DVE+ScalarE keep dq=2 PE-bound.
            # Later groups are >=3 d-tiles so the SBUF-accumulate adds (DVE
            # only) stay cheaper than the PE chain time. ----
            groups = [2, 3, 4, 3, 4]
            offs = [sum(groups[:i]) for i in range(len(groups))]
            comb_n = [0]
            with (
                tc.tile_pool(name="xw", bufs=7) as xw,
                tc.tile_pool(name="pp", bufs=3, space="PSUM") as ppool,
            ):
                for qg, (off, dq) in enumerate(zip(offs, groups)):
                    last = qg == len(groups) - 1
                    xts, wqks, wvs = [], [], []
                    for k in range(dq):
                        di = off + k
                        wv_t = xw.tile([128, NH * HD], DT, tag="wv", bufs=6)
                        nc.sync.dma_start(wv_t[:],
                                          wqkv[di * 128:(di + 1) * 128,
                                               2 * NH * HD:3 * NH * HD])
                        xt_t = xw.tile([128, T], DT, tag="xt")
                        nc.sync.dma_start(xt_t[:], xT[di * 128:(di + 1) * 128, :])
                        wqk_t = xw.tile([128, 2 * NH * HD], DT, tag="wqk")
                        nc.sync.dma_start(wqk_t[:],
                                          wqkv[di * 128:(di + 1) * 128, 0:2 * NH * HD])
                        xts.append(xt_t)
                        wqks.append(wqk_t)
                        wvs.append(wv_t)

                    def acc(ps, dst):
                        comb_n[0] += 1
                        if qg == 0:
                            if comb_n[0] % 2 == 0:
                                nc.scalar.copy(dst, ps[:])
                            else:
                                nc.vector.tensor_copy(dst, ps[:])
                        else:
                            nc.vector.tensor_add(dst, dst, ps[:])

                    # V first: attention's PV chains need V earliest
                    for tt in range(NTT):
                        ps = ppool.tile([128, TCH], F32, tag="pp")
                        for k in range(dq):
                            nc.tensor.matmul(
                                ps[:],
                                xts[k][:, tt * 128:(tt + 1) * 128],
                                wvs[k][:],
                                start=(k == 0),
                                stop=(k == dq - 1),
                            )
                        acc(ps, vsb[:, tt, :])
                    # Q^T / K^T chains; on the last group emit chunks 1,0
                    # first (attention starts at chunk 1)
                    tchs = (1, 0, 2, 3) if last else (0, 1, 2, 3)
                    for tch in tchs:
                        for h in range(NH):
                            for sl, wof in ((0, NH * HD + h * HD), (1, h * HD)):
                                ps = ppool.tile([128, TCH], F32, tag="pp")
                                for k in range(dq):
                                    nc.tensor.matmul(
                                        ps[:],
                                        wqks[k][:, wof:wof + HD],
                                        xts[k][:, tch * TCH:(tch + 1) * TCH],
                                        start=(k == 0),
                                        stop=(k == dq - 1),
                                    )
                                acc(ps, qkt[:, 2 * h + sl,
                                            tch * TCH:(tch + 1) * TCH])
                    if qg == 0:
                        # masks on the scalar HWDGE ring, issued after group
                        # 0's copies: the transfer lands mid-proj without
                        # contending with the first d-tiles, and the tile is
                        # persistent so nothing waits on SBUF reuse
                        nc.scalar.dma_start(msb[:], masks[:])

            # ---- Phase B+C: attention per chunk, with the previous chunk's
            # wo-projection chains interleaved into the jt loop so the
            # in-order PE never sits on PSUM slot recycling ----
            with (
                tc.tile_pool(name="post", bufs=1) as post,
                tc.tile_pool(name="work", bufs=11) as work,
                tc.tile_pool(name="sml", bufs=2) as sml,
                tc.tile_pool(name="otp", bufs=2) as otp,
                tc.tile_pool(name="outp", bufs=4) as outp,
                tc.tile_pool(name="ps3", bufs=3, space="PSUM") as ps3,
                tc.tile_pool(name="pso", bufs=4, space="PSUM") as pso4,
                tc.tile_pool(name="psd2", bufs=1, space="PSUM") as ps1,
            ):
                # wo tiles land in the freed xw region; their dma_starts ride
                # the sync ring (idle until the first out-store ~15us later)
                # so the slot-free wait never blocks the scalar exp stream
                wosb = []
                for et in range(NH):
                    wt_ = post.tile([128, D], DT, tag=f"wos{et}")
                    nc.sync.dma_start(wt_[:], wo[et * 128:(et + 1) * 128, :])
                    wosb.append(wt_)

                pc_n = [0]

                def emit_pc_chain(c0, lt, oc, otc0, alternate=False):
                    if alternate and pc_n[0] % 2 == 0:
                        ps = ps3.tile([128, TCH], F32, tag="ps_s")
                    else:
                        ps = pso4.tile([128, TCH], F32, tag="ps_o")
                    for h2 in range(NH):
                        nc.tensor.matmul(
                            ps[:],
                            otc0[:, h2, lt * 128:(lt + 1) * 128],
                            wosb[h2][:, oc * TCH:(oc + 1) * TCH],
                            start=(h2 == 0),
                            stop=(h2 == NH - 1),
                        )
                    ost = outp.tile([128, TCH], F32, tag="ost")
                    pc_n[0] += 1
                    if pc_n[0] % 2 == 0:
                        nc.scalar.copy(ost[:], ps[:])
                    else:
                        nc.vector.tensor_copy(ost[:], ps[:])
                    deng = nc.scalar if (alternate and pc_n[0] % 2 == 0) \
                        else nc.sync
                    deng.dma_start(
                        out[(4 * c0 + lt) * 128:(4 * c0 + lt + 1) * 128,
                            oc * TCH:(oc + 1) * TCH],
                        ost[:],
                    )

                pending = []
                for c in (1, 2, 3, 0):
                    otc = otp.tile([128, NH, TCH], DT, tag="ot")
                    njt = 4 * (c + 1) if causal else NTT
                    steps_total = NH * njt
                    spacing = max(1, steps_total // len(pending)) if pending else 0
                    step = 0
                    for h in range(NH):
                        pso = pso4.tile([128, TCH], F32, tag="ps_o")
                        psd = ps1.tile([1, TCH], F32, tag="ps_d")

                        def emit_pss(jt):
                            # diagonal tiles: queries < 128*qd are fully
                            # masked for this key tile -> narrow the moving
                            # operand and all downstream elementwise work
                            qd = jt - 4 * c
                            o = qd * 128 if (causal and qd > 0) else 0
                            pss = ps3.tile([128, TCH], F32, tag="ps_s")
                            nc.tensor.matmul(
                                pss[:, o:],
                                qkt[:, 2 * h, jt * 128:(jt + 1) * 128],
                                qkt[:, 2 * h + 1, c * TCH + o:(c + 1) * TCH],
                                start=True,
                                stop=True,
                            )
                            pt = work.tile([128, TCH], DT, tag="pt")
                            nc.scalar.activation(pt[:, o:], pss[:, o:], EXP,
                                                 scale=scale)
                            if causal:
                                if qd >= 0:
                                    nc.vector.tensor_mul(
                                        pt[:, o:], pt[:, o:],
                                        msb[:, qd * TCH + o:(qd + 1) * TCH])
                            else:
                                mt = work.tile([128, TCH], DT, tag="mt")
                                nc.sync.dma_start(
                                    mt[:],
                                    maskT[jt * 128:(jt + 1) * 128,
                                          c * TCH:(c + 1) * TCH],
                                )
                                nc.vector.tensor_mul(pt[:], pt[:], mt[:])
                            return pt, o

                        pts = {}
                        for jt in range(min(3, njt)):
                            pts[jt] = emit_pss(jt)
                        prev_pt = prev_off = None
                        dpend = []   # pair sums awaiting their ones-matmul
                        dn = [0]

                        def flush_den(stop):
                            fpt, foff, _ = dpend.pop(0)
                            nc.tensor.matmul(
                                psd[:, foff:],
                                ones[:, 0:1],
                                fpt[:, foff:],
                                start=(dn[0] == 0),
                                stop=stop,
                            )
                            dn[0] += 1

                        for jt in range(njt):
                            if jt + 3 < njt:
                                pts[jt + 3] = emit_pss(jt + 3)
                            pt, o = pts.pop(jt)
                            nc.tensor.matmul(
                                pso[:, o:],
                                vsb[:, jt, h * HD:(h + 1) * HD],
                                pt[:, o:],
                                start=(jt == 0),
                                stop=(jt == njt - 1),
                            )
                            # denominator: sum pt pairs in place on DVE (the
                            # odd tile's valid region is a suffix of the even
                            # tile's) and run the ones-matmul one pair late so
                            # the DVE add latency never stalls the PE
                            if jt % 2 == 0:
                                prev_pt, prev_off = pt, o
                            else:
                                nc.vector.tensor_add(prev_pt[:, o:],
                                                     prev_pt[:, o:], pt[:, o:])
                                if (o == 0 and dpend and dpend[-1][1] == 0
                                        and not dpend[-1][2]):
                                    # quad-merge full-width pair sums: one
                                    # more DVE add (issued eagerly, so the
                                    # lagged ones-matmul never waits on it),
                                    # one fewer ones-matmul
                                    qt = dpend[-1][0]
                                    nc.gpsimd.tensor_add(qt[:], qt[:],
                                                         prev_pt[:])
                                    dpend[-1][2] = True
                                else:
                                    dpend.append([prev_pt, prev_off, False])
                                if len(dpend) > 1:
                                    flush_den(stop=False)
                            step += 1
                            if pending and spacing and step % spacing == 0:
                                emit_pc_chain(*pending.pop(0))
                        while dpend:
                            flush_den(stop=(len(dpend) == 1))
                        # 1/d via single-op approx reciprocal (~18 bits, way
                        # beyond the ~12-bit fp32r pipeline); exact reciprocal
                        # costs 3.3us and Ln/Exp thrash the ACT table
                        drc = sml.tile([1, TCH], F32, tag="drc")
                        nc.vector.reciprocal_approx_fast(drc[:], psd[:])
                        bc = sml.tile([128, TCH], F32, tag="bc")
                        nc.gpsimd.partition_broadcast(bc[:], drc[:])
                        nc.vector.tensor_mul(otc[:, h, :], pso[:], bc[:])
                    while pending:
                        emit_pc_chain(*pending.pop(0))
                    pending = [(c, lt, oc, otc)
                               for lt in range(4) for oc in range(NCH)]
                # tail drain: alternate the PSUM->SBUF copies between DVE and
                # ScalarE so slot recycling isn't single-engine-latency-bound
                for chain in pending:
                    emit_pc_chain(*chain, alternate=True)
    nc.compile()
    return nc


def _get_built(causal: bool):
    if causal not in _BUILT:
        _BUILT[causal] = _build(causal)
    return _BUILT[causal]


def _diag_masks():
    # masks[jl, q*TCH + ii] = 1 if key (128*q + jl) <= query ii in the chunk
    q = np.arange(4)[:, None, None]
    jl = np.arange(128)[None, :, None]
    ii = np.arange(TCH)[None, None, :]
    m = (ii >= 128 * q + jl).astype(np.float32)        # [4, 128, TCH]
    return np.ascontiguousarray(m.transpose(1, 0, 2).reshape(128, 4 * TCH))


def kernel(x, mask, wqkv, wo):
    global LAST_RESULTS
    from concourse.bass_utils import run_bass_kernel_spmd

    if BF16:
        import ml_dtypes
        sdt = ml_dtypes.bfloat16
    else:
        sdt = np.float32

    x = np.ascontiguousarray(np.asarray(x, dtype=np.float32))
    wqkv = np.asarray(wqkv, dtype=np.float32)
    wo_f = np.ascontiguousarray(np.asarray(wo, dtype=np.float32).astype(sdt))
    mask_np = np.asarray(mask).reshape(T, T).astype(bool)
    causal = bool(np.array_equal(mask_np, np.tril(np.ones((T, T), dtype=bool))))

    nc = _get_built(causal)
    masks_arr = _diag_masks().astype(sdt)
    maskT = None
    if not causal:
        maskT = np.ascontiguousarray(mask_np.T.astype(sdt))

    in_maps = []
    for core in range(NCORES):
        b, g = divmod(core, NH)
        xT = np.ascontiguousarray(x[b].T.astype(sdt))
        wq = wqkv[:, 0 * H * HD + g * NH * HD:0 * H * HD + (g + 1) * NH * HD]
        wk = wqkv[:, 1 * H * HD + g * NH * HD:1 * H * HD + (g + 1) * NH * HD]
        wv = wqkv[:, 2 * H * HD + g * NH * HD:2 * H * HD + (g + 1) * NH * HD]
        wqkv_g = np.ascontiguousarray(
            np.concatenate([wq, wk, wv], axis=1).astype(sdt))
        wo_g = np.ascontiguousarray(wo_f[g * NH * HD:(g + 1) * NH * HD, :])
        m = {"xT": xT, "wqkv": wqkv_g, "wo": wo_g, "masks": masks_arr}
        if maskT is not None:
            m["maskT"] = maskT
        in_maps.append(m)

    trace = os.environ.get("ATTN_TRACE", "") not in ("", "0")
    res = run_bass_kernel_spmd(nc, in_maps, core_ids=list(range(NCORES)),
                               trace=trace)
    LAST_RESULTS = res

    acc = np.zeros((B, T, D), dtype=np.float64)
    for core in range(NCORES):
        b = core // NH
        acc[b] += res.results[core]["out"].astype(np.float64)
    return acc.astype(np.float32)


# revision 14
# speedup vs baseline: 1.4713x; 1.4713x over previous
"""Causal multi-head attention (B=2, T=2048, DIM=2048, H=16, HD=128) on 8
Trainium2 NeuronCores.

Sharding: core = 4*b + g  (b = batch 0..1, g = head-group 0..3, 4 heads each).
Each core computes, for its batch b and heads 4g..4g+3:
  QKV projection -> causal attention -> partial out = attn_out @ wo[rows of g]
The host sums the 4 partial outputs per batch (the "all-reduce after wo").

On-device layout avoids every transpose:
  - host passes x[b].T, so projections contract d with d on partitions
  - Q^T/K^T kept as [hd, t] (head dim on partitions): K_h at qkt slot 2h,
    Q_h at slot 2h+1
  - scores computed as S^T = K^T_tile.T @ Q^T  ([j, i] layout)
  - exp via ScalarE; causal masking = multiply diagonal tiles by 0/1 masks
  - diagonal score tiles are NARROWED: only query columns >= key-tile start
    are computed/exp'd/masked (the rest is fully masked anyway)
  - P@V computed as O^T via lhsT = V tile (natural [t, hd] layout)
  - denominator via ones-vector matmul over in-place pair sums on DVE,
    lagged one pair so the add latency stays off the in-order PE
  - normalization via approx reciprocal + GPSIMD partition-broadcast of 1/d
    + VectorE multiply
  - wo projection consumes O^T tiles directly as stationary operands;
    chunk order 1,2,3,0 so every chunk's wo chains interleave into the next
    chunk's score/PV stream and the tail drain is short
Projection streams d-tiles in groups [2,3,4,4,3] (compute starts after two
d-tiles; group-0 PSUM->SBUF copies alternate DVE/ScalarE so dq=2 stays
PE-bound); per d-tile DMA order is wv, xT, wqk so the first V chain starts
earliest. masks/wo loads ride the scalar HWDGE ring, issued between groups
0 and 1 so they never contend with the first d-tiles or attention.

Weights/activations stream as bfloat16 (PE runs bf16 at the same full rate
as float32r; inputs/outputs of every matmul keep consistent dtype; psum,
denominators, reciprocal, and the final output stay fp32). End-to-end rel
err ~1e-3 vs the 2e-2 gate. ATTN_BF16=0 falls back to float32r.
"""

import math
import os

import numpy as np

B, T, D, H, HD = 2, 2048, 2048, 16, 128
NH = 4            # heads per core
NCORES = 8
TCH = 512         # query-chunk width (moving-operand free size)
NDT = D // 128    # 16 d-tiles (contraction tiles for projections)
NTT = T // 128    # 16 t-tiles
NCH = T // TCH    # 4 query chunks

BF16 = os.environ.get("ATTN_BF16", "0") not in ("", "0")

_BUILT = {}
LAST_RESULTS = None  # BassKernelResults of the most recent kernel() call


def _build(causal: bool):
    import concourse.mybir as mybir
    import concourse.tile as tile
    from concourse import bacc

    F32 = mybir.dt.float32
    DT = mybir.dt.bfloat16 if BF16 else mybir.dt.float32r
    EXP = mybir.ActivationFunctionType.Exp
    scale = 1.0 / math.sqrt(HD)

    nc = bacc.Bacc(None, name="attn")
    xT = nc.dram_tensor("xT", [D, T], DT, kind="ExternalInput")
    wqkv = nc.dram_tensor("wqkv", [D, 3 * NH * HD], DT, kind="ExternalInput")
    wo = nc.dram_tensor("wo", [NH * HD, D], DT, kind="ExternalInput")
    masks = nc.dram_tensor("masks", [128, 4 * TCH], DT, kind="ExternalInput")
    if not causal:
        maskT = nc.dram_tensor("maskT", [T, T], DT, kind="ExternalInput")
    out = nc.dram_tensor("out", [T, D], F32, kind="ExternalOutput")

    with tile.TileContext(nc) as tc:
        with tc.tile_pool(name="persist", bufs=1) as persist:
            # persistent operands for the attention phase
            qkt = persist.tile([128, 8, T], DT)           # slot 2h: K_h, 2h+1: Q_h
            vsb = persist.tile([128, NTT, NH * HD], DT)   # V, [t-tile][local t, head*hd]
            msb = persist.tile([128, 4 * TCH], DT)        # diagonal causal masks
            ones_f = persist.tile([128, 1], F32)
            ones = persist.tile([128, 1], DT)
            nc.vector.memset(ones_f[:], 1.0)
            nc.vector.tensor_copy(ones[:], ones_f[:])
            # dummy broadcast: preload the GpSimd PartitionBroadcast ucode
            # library now (~11us HBM fetch) so the first real normalize
            # doesn't stall the whole attention pipeline on LIBRARY_RELOAD
            warm = persist.tile([128, 1], F32)
            nc.gpsimd.partition_broadcast(warm[:], ones_f[0:1, :])

            # ---- Phase A: QKV projections, streaming x^T / wqkv d-tiles.
            # groups[0]=2 so compute starts after ~2.5MB of DMA; its
            # PSUM->SBUF copies split across DVE+ScalarE keep dq=2 PE-bound.
            # Later groups are >=3 d-tiles so the SBUF-accumulate adds (DVE
            # only) stay cheaper than the PE chain time. ----
            groups = [2, 3, 4, 3, 4]
            offs = [sum(groups[:i]) for i in range(len(groups))]
            comb_n = [0]
            with (
                tc.tile_pool(name="xw", bufs=7) as xw,
                tc.tile_pool(name="pp", bufs=3, space="PSUM") as ppool,
            ):
                for qg, (off, dq) in enumerate(zip(offs, groups)):
                    last = qg == len(groups) - 1
                    xts, wqks, wvs = [], [], []
                    for k in range(dq):
                        di = off + k
                        wv_t = xw.tile([128, NH * HD], DT, tag="wv", bufs=6)
                        nc.sync.dma_start(wv_t[:],
                                          wqkv[di * 128:(di + 1) * 128,
                                               2 * NH * HD:3 * NH * HD])
                        xt_t = xw.tile([128, T], DT, tag="xt")
                        nc.sync.dma_start(xt_t[:], xT[di * 128:(di + 1) * 128, :])
                        wqk_t = xw.tile([128, 2 * NH * HD], DT, tag="wqk")
                        nc.sync.dma_start(wqk_t[:],
                                          wqkv[di * 128:(di + 1) * 128, 0:2 * NH * HD])
                        xts.append(xt_t)
                        wqks.append(wqk_t)
                        wvs.append(wv_t)

                    def acc(ps, dst):
                        comb_n[0] += 1
                        if qg == 0:
                            if comb_n[0] % 2 == 0:
                                nc.scalar.copy(dst, ps[:])
                            else:
                                nc.vector.tensor_copy(dst, ps[:])
                        else:
                            nc.vector.tensor_add(dst, dst, ps[:])

                    # V first: attention's PV chains need V earliest
                    for tt in range(NTT):
                        ps = ppool.tile([128, TCH], F32, tag="pp")
                        for k in range(dq):
                            nc.tensor.matmul(
                                ps[:],
                                xts[k][:, tt * 128:(tt + 1) * 128],
                                wvs[k][:],
                                start=(k == 0),
                                stop=(k == dq - 1),
                            )
                        acc(ps, vsb[:, tt, :])
                    # Q^T / K^T chains; on the last group emit chunks 1,0
                    # first (attention starts at chunk 1)
                    tchs = (1, 0, 2, 3) if last else (0, 1, 2, 3)
                    for tch in tchs:
                        for h in range(NH):
                            for sl, wof in ((0, NH * HD + h * HD), (1, h * HD)):
                                ps = ppool.tile([128, TCH], F32, tag="pp")
                                for k in range(dq):
                                    nc.tensor.matmul(
                                        ps[:],
                                        wqks[k][:, wof:wof + HD],
                                        xts[k][:, tch * TCH:(tch + 1) * TCH],
                                        start=(k == 0),
                                        stop=(k == dq - 1),
                                    )
                                acc(ps, qkt[:, 2 * h + sl,
                                            tch * TCH:(tch + 1) * TCH])
                    if qg == 0:
                        # masks on the scalar HWDGE ring, issued after group
                        # 0's copies: the transfer lands mid-proj without
                        # contending with the first d-tiles, and the tile is
                        # persistent so nothing waits on SBUF reuse
                        nc.scalar.dma_start(msb[:], masks[:])

            # ---- Phase B+C: attention per chunk, with the previous chunk's
            # wo-projection chains interleaved into the jt loop so the
            # in-order PE never sits on PSUM slot recycling ----
            with (
                tc.tile_pool(name="post", bufs=1) as post,
                tc.tile_pool(name="work", bufs=11) as work,
                tc.tile_pool(name="sml", bufs=2) as sml,
                tc.tile_pool(name="otp", bufs=2) as otp,
                tc.tile_pool(name="outp", bufs=4) as outp,
                tc.tile_pool(name="ps3", bufs=3, space="PSUM") as ps3,
                tc.tile_pool(name="pso", bufs=4, space="PSUM") as pso4,
                tc.tile_pool(name="psd2", bufs=1, space="PSUM") as ps1,
            ):
                # wo tiles land in the freed xw region; their dma_starts ride
                # the sync ring (idle until the first out-store ~15us later)
                # so the slot-free wait never blocks the scalar exp stream
                wosb = []
                for et in range(NH):
                    wt_ = post.tile([128, D], DT, tag=f"wos{et}")
                    nc.sync.dma_start(wt_[:], wo[et * 128:(et + 1) * 128, :])
                    wosb.append(wt_)

                pc_n = [0]

                def emit_pc_chain(c0, lt, oc, otc0, alternate=False):
                    if alternate and pc_n[0] % 2 == 0:
                        ps = ps3.tile([128, TCH], F32, tag="ps_s")
                    else:
                        ps = pso4.tile([128, TCH], F32, tag="ps_o")
                    for h2 in range(NH):
                        nc.tensor.matmul(
                            ps[:],
                            otc0[:, h2, lt * 128:(lt + 1) * 128],
                            wosb[h2][:, oc * TCH:(oc + 1) * TCH],
                            start=(h2 == 0),
                            stop=(h2 == NH - 1),
                        )
                    ost = outp.tile([128, TCH], F32, tag="ost")
                    pc_n[0] += 1
                    if pc_n[0] % 2 == 0:
                        nc.scalar.copy(ost[:], ps[:])
                    else:
                        nc.vector.tensor_copy(ost[:], ps[:])
                    deng = nc.scalar if (alternate and pc_n[0] % 2 == 0) \
                        else nc.sync
                    deng.dma_start(
                        out[(4 * c0 + lt) * 128:(4 * c0 + lt + 1) * 128,
                            oc * TCH:(oc + 1) * TCH],
                        ost[:],
                    )

                pending = []
                for c in (1, 2, 3, 0):
                    otc = otp.tile([128, NH, TCH], DT, tag="ot")
                    njt = 4 * (c + 1) if causal else NTT
                    steps_total = NH * njt
                    spacing = max(1, steps_total // len(pending)) if pending else 0
                    step = 0
                    for h in range(NH):
                        pso = pso4.tile([128, TCH], F32, tag="ps_o")
                        psd = ps1.tile([1, TCH], F32, tag="ps_d")

                        def emit_pss(jt):
                            # diagonal tiles: queries < 128*qd are fully
                            # masked for this key tile -> narrow the moving
                            # operand and all downstream elementwise work
                            qd = jt - 4 * c
                            o = qd * 128 if (causal and qd > 0) else 0
                            pss = ps3.tile([128, TCH], F32, tag="ps_s")
                            nc.tensor.matmul(
                                pss[:, o:],
                                qkt[:, 2 * h, jt * 128:(jt + 1) * 128],
                                qkt[:, 2 * h + 1, c * TCH + o:(c + 1) * TCH],
                                start=True,
                                stop=True,
                            )
                            pt = work.tile([128, TCH], DT, tag="pt")
                            nc.scalar.activation(pt[:, o:], pss[:, o:], EXP,
                                                 scale=scale)
                            if causal:
                                if qd >= 0:
                                    # the mask only zeroes the 128-wide
                                    # strip at the diagonal (keys span 128
                                    # rows), so the mul covers just that strip
                                    nc.vector.tensor_mul(
                                        pt[:, o:o + 128], pt[:, o:o + 128],
                                        msb[:, qd * TCH + o:qd * TCH + o + 128])
                            else:
                                mt = work.tile([128, TCH], DT, tag="mt")
                                nc.sync.dma_start(
                                    mt[:],
                                    maskT[jt * 128:(jt + 1) * 128,
                                          c * TCH:(c + 1) * TCH],
                                )
                                nc.vector.tensor_mul(pt[:], pt[:], mt[:])
                            return pt, o

                        pts = {}
                        for jt in range(min(3, njt)):
                            pts[jt] = emit_pss(jt)
                        prev_pt = prev_off = None
                        dpend = []   # pair sums awaiting their ones-matmul
                        dn = [0]

                        def flush_den(stop):
                            fpt, foff, _ = dpend.pop(0)
                            nc.tensor.matmul(
                                psd[:, foff:],
                                ones[:, 0:1],
                                fpt[:, foff:],
                                start=(dn[0] == 0),
                                stop=stop,
                            )
                            dn[0] += 1

                        for jt in range(njt):
                            if jt + 3 < njt:
                                pts[jt + 3] = emit_pss(jt + 3)
                            pt, o = pts.pop(jt)
                            nc.tensor.matmul(
                                pso[:, o:],
                                vsb[:, jt, h * HD:(h + 1) * HD],
                                pt[:, o:],
                                start=(jt == 0),
                                stop=(jt == njt - 1),
                            )
                            # denominator: sum pt pairs in place on DVE (the
                            # odd tile's valid region is a suffix of the even
                            # tile's) and run the ones-matmul one pair late so
                            # the DVE add latency never stalls the PE
                            if jt % 2 == 0:
                                prev_pt, prev_off = pt, o
                            else:
                                nc.vector.tensor_add(prev_pt[:, o:],
                                                     prev_pt[:, o:], pt[:, o:])
                                if (o == 0 and dpend and dpend[-1][1] == 0
                                        and not dpend[-1][2]):
                                    # quad-merge full-width pair sums: one
                                    # more DVE add (issued eagerly, so the
                                    # lagged ones-matmul never waits on it),
                                    # one fewer ones-matmul
                                    qt = dpend[-1][0]
                                    nc.vector.tensor_add(qt[:], qt[:],
                                                         prev_pt[:])
                                    dpend[-1][2] = True
                                else:
                                    dpend.append([prev_pt, prev_off, False])
                                if len(dpend) > 1:
                                    flush_den(stop=False)
                            step += 1
                            if pending and spacing and step % spacing == 0:
                                emit_pc_chain(*pending.pop(0))
                        while dpend:
                            flush_den(stop=(len(dpend) == 1))
                        # 1/d via single-op approx reciprocal (~18 bits, way
                        # beyond the ~12-bit fp32r pipeline); exact reciprocal
                        # costs 3.3us and Ln/Exp thrash the ACT table
                        drc = sml.tile([1, TCH], F32, tag="drc")
                        nc.vector.reciprocal_approx_fast(drc[:], psd[:])
                        bc = sml.tile([128, TCH], F32, tag="bc")
                        nc.gpsimd.partition_broadcast(bc[:], drc[:])
                        nc.vector.tensor_mul(otc[:, h, :], pso[:], bc[:])
                    while pending:
                        emit_pc_chain(*pending.pop(0))
                    pending = [(c, lt, oc, otc)
                               for lt in range(4) for oc in range(NCH)]
                # tail drain: alternate the PSUM->SBUF copies between DVE and
                # ScalarE so slot recycling isn't single-engine-latency-bound
                for chain in pending:
                    emit_pc_chain(*chain, alternate=True)
    nc.compile()
    return nc


def _get_built(causal: bool):
    if causal not in _BUILT:
        _BUILT[causal] = _build(causal)
    return _BUILT[causal]


def _diag_masks():
    # masks[jl, q*TCH + ii] = 1 if key (128*q + jl) <= query ii in the chunk
    q = np.arange(4)[:, None, None]
    jl = np.arange(128)[None, :, None]
    ii = np.arange(TCH)[None, None, :]
    m = (ii >= 128 * q + jl).astype(np.float32)        # [4, 128, TCH]
    return np.ascontiguousarray(m.transpose(1, 0, 2).reshape(128, 4 * TCH))


def kernel(x, mask, wqkv, wo):
    global LAST_RESULTS
    from concourse.bass_utils import run_bass_kernel_spmd

    if BF16:
        import ml_dtypes
        sdt = ml_dtypes.bfloat16
    else:
        sdt = np.float32

    x = np.ascontiguousarray(np.asarray(x, dtype=np.float32))
    wqkv = np.asarray(wqkv, dtype=np.float32)
    wo_f = np.ascontiguousarray(np.asarray(wo, dtype=np.float32).astype(sdt))
    mask_np = np.asarray(mask).reshape(T, T).astype(bool)
    causal = bool(np.array_equal(mask_np, np.tril(np.ones((T, T), dtype=bool))))

    nc = _get_built(causal)
    masks_arr = _diag_masks().astype(sdt)
    maskT = None
    if not causal:
        maskT = np.ascontiguousarray(mask_np.T.astype(sdt))

    in_maps = []
    for core in range(NCORES):
        b, g = divmod(core, NH)
        xT = np.ascontiguousarray(x[b].T.astype(sdt))
        wq = wqkv[:, 0 * H * HD + g * NH * HD:0 * H * HD + (g + 1) * NH * HD]
        wk = wqkv[:, 1 * H * HD + g * NH * HD:1 * H * HD + (g + 1) * NH * HD]
        wv = wqkv[:, 2 * H * HD + g * NH * HD:2 * H * HD + (g + 1) * NH * HD]
        wqkv_g = np.ascontiguousarray(
            np.concatenate([wq, wk, wv], axis=1).astype(sdt))
        wo_g = np.ascontiguousarray(wo_f[g * NH * HD:(g + 1) * NH * HD, :])
        m = {"xT": xT, "wqkv": wqkv_g, "wo": wo_g, "masks": masks_arr}
        if maskT is not None:
            m["maskT"] = maskT
        in_maps.append(m)

    trace = os.environ.get("ATTN_TRACE", "") not in ("", "0")
    res = run_bass_kernel_spmd(nc, in_maps, core_ids=list(range(NCORES)),
                               trace=trace)
    LAST_RESULTS = res

    acc = np.zeros((B, T, D), dtype=np.float64)
    for core in range(NCORES):
        b = core // NH
        acc[b] += res.results[core]["out"].astype(np.float64)
    return acc.astype(np.float32)
